# revision 1
# baseline (speedup 1.0000x reference)
"""Trainium2 Bass kernel for nn_BinaryConnectNet (binary CNN, 8 NeuronCores).

Sharding: batch-parallel convs (128 img/core), fc1 output-feature-sharded
(128 features/core) with an on-device AllGather of the binary activations
(fp8, values +-1). fc2 computed as per-core partials, summed on host.

Numerics:
 - conv1 (dw3x3 + 1x1 fused into a dense 3x3 with +-1 weights): input x is
   triple-bf16-split (exact to 2^-24) and contracted in one K=81 matmul per
   output tile; sign()+bias is folded into PSUM eviction; 2x2 maxpool is a
   max-tree over 4 quadrant matmul passes.
 - conv2 depthwise: 9 PSUM-accumulated diagonal matmuls over shifted windows
   of the zero-padded +-1 activations (integer-exact in bf16).
 - conv2 pointwise: dense K=128 matmul, +-1 weights, integer-exact.
 - fc1: weights split hi/lo fp16 (exact to 2^-22), both accumulated into one
   PSUM; rhs is the gathered +-1 activations in fp8.
 - fc2: fp16 hi/lo, per-core partial output in fp32.
"""

import sys

for _p in ("/opt/trn_rl_repo",):
    if _p not in sys.path:
        sys.path.insert(0, _p)

import numpy as np
import ml_dtypes
from contextlib import ExitStack

import concourse.bass as bass
import concourse.bacc as bacc
import concourse.mybir as mybir
import concourse.tile as tile
from concourse.bass_utils import run_bass_kernel_spmd

F32 = mybir.dt.float32
BF16 = mybir.dt.bfloat16
FP16 = mybir.dt.float16
FP8 = mybir.dt.float8e4
AF = mybir.ActivationFunctionType
ALU = mybir.AluOpType

NCORES = 8
B = 128               # images per core
H = 32                # conv1 spatial
HP = 34               # padded
ROWLEN = B * HP       # 4352: one padded h-row across batch (b, w) flattened
X9_SLACK = 8
X9_ROW = HP * ROWLEN + X9_SLACK   # flattened (h, b, w) per (c, s) row + slack
P1 = 16               # pooled spatial after pool1
P1PAD = 18
P2 = 8                # pooled spatial after pool2
NF1 = 1024            # fc1 features (global)
FPC = NF1 // NCORES   # fc1 features per core = 128
KFC = 256 * P2 * P2   # fc1 contraction = 16384
NKT = KFC // 128      # 128 K-tiles
NB_ALL = NCORES * B   # 1024


def _bf16(a):
    return np.asarray(a, dtype=ml_dtypes.bfloat16)


def _host_prep(x, w1_dw, b1_dw, w1_pw, b1_pw, w2_dw, b2_dw, w2_pw, b2_pw,
               fc1_w, fc1_b, fc2_w, fc2_b, ncores=NCORES, nb=B):
    """Build all per-core device input arrays (numpy only)."""
    sgn = np.sign
    x = np.asarray(x, np.float32)
    rowlen = nb * HP
    x9row = HP * rowlen + X9_SLACK
    fpc = FPC

    # triple bf16 split of x
    x0 = _bf16(x)
    r1 = x - x0.astype(np.float32)
    x1 = _bf16(r1)
    r2 = r1 - x1.astype(np.float32)
    x2 = _bf16(r2)
    splits = [x0, x1, x2]

    # x9h: per core [9 rows (3c+s), x9row] bf16 with (h, b, w) layout, pad 1
    x9h = np.zeros((ncores, 9, x9row), dtype=ml_dtypes.bfloat16)
    for s in range(3):
        xs = splits[s].reshape(ncores, nb, 3, H, H)
        for c in range(3):
            # [nb, H, H] -> padded (HP, nb, HP)
            row = np.zeros((ncores, HP, nb, HP), dtype=ml_dtypes.bfloat16)
            row[:, 1:33, :, 1:33] = xs[:, :, c].transpose(0, 2, 1, 3)
            x9h[:, 3 * c + s, : HP * rowlen] = row.reshape(ncores, -1)

    # conv1 fused weights: lhsT [81, 128]
    s1dw = sgn(np.asarray(w1_dw, np.float32))[:, 0]       # [3, 3, 3]
    s1pw = sgn(np.asarray(w1_pw, np.float32))[:, :, 0, 0]  # [128, 3]
    w1t = np.zeros((81, 128), dtype=ml_dtypes.bfloat16)
    for du in range(3):
        for dv in range(3):
            for c in range(3):
                for s in range(3):
                    w1t[9 * (3 * du + dv) + 3 * c + s] = _bf16(
                        s1pw[:, c] * s1dw[c, du, dv])
    b1eff = (sgn(np.asarray(b1_pw, np.float32))
             + s1pw @ sgn(np.asarray(b1_dw, np.float32))).astype(np.float32)

    # conv2 depthwise: 9 diagonal lhsT [128, 9*128]
    s2dw = sgn(np.asarray(w2_dw, np.float32))[:, 0]       # [128, 3, 3]
    dwt = np.zeros((128, 9 * 128), dtype=ml_dtypes.bfloat16)
    for du in range(3):
        for dv in range(3):
            t = 3 * du + dv
            np.fill_diagonal(dwt[:, 128 * t:128 * (t + 1)],
                             _bf16(s2dw[:, du, dv]))
    dwb = sgn(np.asarray(b2_dw, np.float32)).astype(np.float32)  # [128]

    # conv2 pointwise lhsT [128, 256] and bias
    s2pw = sgn(np.asarray(w2_pw, np.float32))[:, :, 0, 0]  # [256, 128]
    pwt = _bf16(s2pw.T)                                    # [128, 256]
    b2m = sgn(np.asarray(b2_pw, np.float32)).astype(np.float32)  # [256]

    # fc1 hi/lo fp16, column-permuted to device K-tile order, per-core slice
    fc1_w = np.asarray(fc1_w, np.float32)                  # [1024, 16384]
    # device feature order: kt = ct*64 + s0, partition c' -> col (ct*128+c')*64+s0
    cols = np.empty(KFC, np.int64)
    i = 0
    for ct in range(2):
        for s0 in range(64):
            for cp in range(128):
                cols[i] = (ct * 128 + cp) * 64 + s0
                i += 1
    wperm = fc1_w[:, cols]                                 # [1024, 16384(dev)]
    whi = wperm.astype(np.float16)
    wlo = (wperm - whi.astype(np.float32)).astype(np.float16)
    # per-core [NKT, 128(c'), 128(o_local)]; core n gets features n*fpc..
    whi_t = whi.reshape(8, fpc, NKT, 128).transpose(0, 2, 3, 1).copy()
    wlo_t = wlo.reshape(8, fpc, NKT, 128).transpose(0, 2, 3, 1).copy()

    # fc2 hi/lo fp16 per-core slice: lhsT [128(f_local), 10]
    fc2_w = np.asarray(fc2_w, np.float32)                  # [10, 1024]
    f2 = fc2_w.T.reshape(8, fpc, 10)
    f2hi = f2.astype(np.float16)
    f2lo = (f2 - f2hi.astype(np.float32)).astype(np.float16)

    shared = {
        "w1t": w1t, "b1eff": b1eff.reshape(128, 1),
        "negb1": (-b1eff).reshape(128, 1).astype(np.float32),
        "dwt": dwt, "dwb": dwb.reshape(128, 1),
        "pwt": pwt, "b2m": b2m.reshape(2, 128).T.copy().astype(np.float32),
        "negb2": (-b2m).reshape(2, 128).T.copy().astype(np.float32),
    }
    per_core = []
    for n in range(ncores):
        d = dict(shared)
        d["x9h"] = x9h[n]
        d["whi"] = whi_t[n]
        d["wlo"] = wlo_t[n]
        d["f2hi"] = f2hi[n]
        d["f2lo"] = f2lo[n]
        per_core.append(d)
    return per_core


def build_program(ncores=NCORES, nb=B, repeats=1):
    """Build the Bass program. nb = images per core. Returns nc."""
    rowlen = nb * HP
    x9row = HP * rowlen + X9_SLACK
    nsh = ncores                    # shards gathered for fc1
    nball = ncores * nb             # total batch
    # fc1 batch chunk: N columns per matmul
    bc_n = min(512, nball)
    nbc = nball // bc_n             # number of batch chunks
    sh_per_bc = bc_n // nb          # shards per batch chunk

    nc = bacc.Bacc("TRN2", target_bir_lowering=False, debug=False,
                   num_devices=ncores)

    def din(name, shape, dt):
        return nc.dram_tensor(name, shape, dt, kind="ExternalInput").ap()

    x9h = din("x9h", [9, x9row], BF16)
    w1t = din("w1t", [81, 128], BF16)
    b1eff = din("b1eff", [128, 1], F32)
    negb1 = din("negb1", [128, 1], F32)
    dwt = din("dwt", [128, 9 * 128], BF16)
    dwb = din("dwb", [128, 1], F32)
    pwt = din("pwt", [128, 256], BF16)
    b2m = din("b2m", [128, 2], F32)
    negb2 = din("negb2", [128, 2], F32)
    whi = din("whi", [NKT, 128, FPC], FP16)
    wlo = din("wlo", [NKT, 128, FPC], FP16)
    f2hi = din("f2hi", [FPC, 10], FP16)
    f2lo = din("f2lo", [FPC, 10], FP16)
    y_out = nc.dram_tensor("y", [10, nball], F32, kind="ExternalOutput").ap()

    # collective bounce buffers (fp8 +-1 activations)
    h2_shard = nc.dram_tensor("h2_shard", [2, 128, nb * 64], FP8).ap()
    h2_all = nc.dram_tensor("h2_all", [nsh, 2, 128, nb * 64], FP8,
                            addr_space="Shared").ap()

    nh2 = H // 2  # 16 pooled rows after pool1

    for _rep in range(repeats):
      with tile.TileContext(nc) as tc, ExitStack() as ctx:
        cpool = ctx.enter_context(tc.tile_pool(name="consts", bufs=1))
        w1_t = cpool.tile([81, 128], BF16)
        nc.sync.dma_start(w1_t[:], w1t[:])
        b1_t = cpool.tile([128, 1], F32)
        nc.sync.dma_start(b1_t[:], b1eff[:])
        nb1_t = cpool.tile([128, 1], F32)
        nc.sync.dma_start(nb1_t[:], negb1[:])
        dw_t = cpool.tile([128, 9 * 128], BF16)
        nc.sync.dma_start(dw_t[:], dwt[:])
        dwb_t = cpool.tile([128, 1], F32)
        nc.sync.dma_start(dwb_t[:], dwb[:])
        pw_t = cpool.tile([128, 256], BF16)
        nc.sync.dma_start(pw_t[:], pwt[:])
        b2_t = cpool.tile([128, 2], F32)
        nc.sync.dma_start(b2_t[:], b2m[:])
        nb2_t = cpool.tile([128, 2], F32)
        nc.sync.dma_start(nb2_t[:], negb2[:])

        # HAM warmup: ~5us of back-to-back matmuls so conv1 runs at 2.4GHz
        with tc.tile_pool(name="warm", bufs=1, space="PSUM") as wps:
            wp_t = wps.tile([128, 512], F32)
            for _w in range(24):
                nc.tensor.matmul(wp_t[:], dw_t[:, 0:128], dw_t[:, 0:512],
                                 start=(_w == 0), stop=(_w == 23))

        sidx = [0]  # round-robin index for sign-evict engine choice

        def sign_evict(ps_ap, out_ap, bias_t, nbias_t):
            """out = sign(psum + bias) as +-1 bf16. Alternates ACT/DVE."""
            k = sidx[0] % 11
            sidx[0] += 1
            if k < 8:
                nc.scalar.activation(out_ap, ps_ap, AF.Sign, bias=bias_t[:])
            else:
                nc.vector.tensor_scalar(out_ap, ps_ap, nbias_t[:], 2.0,
                                        ALU.is_ge, ALU.mult)
                nc.vector.tensor_scalar(out_ap, out_ap, 1.0, None,
                                        ALU.subtract)

        h2pool = ctx.enter_context(tc.tile_pool(name="h2", bufs=1))
        h2t = [h2pool.tile([128, nb * 64], FP8, tag=f"h2_{m}", name=f"h2_{m}")
               for m in range(2)]

        if True:
            with tc.tile_pool(name="h1p", bufs=1) as h1pool:
                # ---- conv1 + pool1 -> h1p (padded, +-1 bf16) ----
                h1p = h1pool.tile([128, nb * P1PAD * P1PAD], BF16)
                h1v = h1p[:].rearrange("p (b y x) -> p b y x",
                                       b=nb, y=P1PAD)
                # zero the pad border
                nc.vector.memset(h1v[:, :, 0, :], 0.0)
                nc.vector.memset(h1v[:, :, P1PAD - 1, :], 0.0)
                nc.vector.memset(h1v[:, :, 1:P1PAD - 1, 0], 0.0)
                nc.vector.memset(h1v[:, :, 1:P1PAD - 1, P1PAD - 1], 0.0)

                with tc.tile_pool(name="c1work", bufs=2) as impool, \
                     tc.tile_pool(name="c1sq", bufs=4) as sqpool, \
                     tc.tile_pool(name="c1tr", bufs=2) as trpool, \
                     tc.tile_pool(name="c1ps", bufs=2, space="PSUM") as pspool:
                    nmm = max(1, (nb * 16) // 512)
                    mmw = min(512, nb * 16) // 16    # images per matmul
                    for hc in range(nh2):
                        imt = impool.tile([81, 2 * rowlen], BF16, tag="im")
                        for du in range(3):
                            for dv in range(3):
                                r0 = 9 * (3 * du + dv)
                                off = (2 * hc + du) * rowlen + dv
                                nc.sync.dma_start(
                                    imt[r0:r0 + 9, :],
                                    x9h[:, off:off + 2 * rowlen])
                        imv = imt[:].rearrange(
                            "p (h b w2 dx) -> p h b w2 dx",
                            h=2, b=nb, w2=HP // 2)
                        sq = []
                        for dy, dx in ((0, 0), (0, 1), (1, 0), (1, 1)):
                            ps = pspool.tile([128, nb * 16], F32, tag="ps")
                            for j in range(nmm):
                                nc.tensor.matmul(
                                    ps[:, j * mmw * 16:(j + 1) * mmw * 16],
                                    w1_t[:],
                                    imv[:, dy, j * mmw:(j + 1) * mmw,
                                        0:16, dx],
                                    start=True, stop=True)
                            sqt = sqpool.tile([128, nb * 16], BF16, tag="sq")
                            sign_evict(ps[:], sqt[:], b1_t, nb1_t)
                            sq.append(sqt)
                        t01 = trpool.tile([128, nb * 16], BF16, tag="tr")
                        t23 = trpool.tile([128, nb * 16], BF16, tag="tr")
                        nc.vector.tensor_max(t01[:], sq[0][:], sq[1][:])
                        nc.vector.tensor_max(t23[:], sq[2][:], sq[3][:])
                        nc.vector.tensor_max(
                            h1v[:, :, hc + 1, 1:P1PAD - 1],
                            t01[:].rearrange("p (b x) -> p b x", b=nb),
                            t23[:].rearrange("p (b x) -> p b x", b=nb))

                # ---- conv2: dw -> pw pipelined per batch chunk ----
                cimg = max(8, min(16, nb))       # images per chunk
                ncch = nb // cimg
                jw = 2                           # images per dw matmul
                jw2 = 8                          # images per pw matmul
                with tc.tile_pool(name="dwch", bufs=2) as dwpool, \
                     tc.tile_pool(name="pwsq", bufs=5) as sq2pool, \
                     tc.tile_pool(name="pwtr", bufs=2) as tr2pool, \
                     tc.tile_pool(name="dwps", bufs=2, space="PSUM") as dps, \
                     tc.tile_pool(name="pwps", bufs=2, space="PSUM") as pps:
                    for g in range(ncch):
                        dwc = dwpool.tile([128, cimg * 256], BF16, tag="dwc")
                        gsub = max(2, min(4, cimg))  # images per dw psum
                        for sub in range(cimg // gsub):
                            ps = dps.tile([128, gsub * 256], F32, tag="dps")
                            for t in range(9):
                                du, dv = t // 3, t % 3
                                for j in range(gsub // jw):
                                    b0 = g * cimg + sub * gsub + j * jw
                                    nc.tensor.matmul(
                                        ps[:, j * jw * 256:(j + 1) * jw * 256],
                                        dw_t[:, 128 * t:128 * (t + 1)],
                                        h1v[:, b0:b0 + jw, du:du + P1,
                                            dv:dv + P1],
                                        start=(t == 0), stop=(t == 8))
                            nc.scalar.activation(
                                dwc[:, sub * gsub * 256:(sub + 1) * gsub * 256],
                                ps[:], AF.Identity, bias=dwb_t[:])
                        dwv = dwc[:].rearrange(
                            "p (b y2 dy x2 dx) -> p b y2 dy x2 dx",
                            b=cimg, y2=P2, dy=2, x2=P2)
                        for mt in range(2):
                            sq2 = []
                            for dy, dx in ((0, 0), (0, 1), (1, 0), (1, 1)):
                                sqt = sq2pool.tile([128, cimg * 64], BF16,
                                                   tag="sq2")
                                ps = pps.tile([128, cimg * 64], F32,
                                              tag="pps")
                                for j in range(cimg // jw2):
                                    nc.tensor.matmul(
                                        ps[:, j * jw2 * 64:(j + 1) * jw2 * 64],
                                        pw_t[:, 128 * mt:128 * (mt + 1)],
                                        dwv[:, j * jw2:(j + 1) * jw2,
                                            :, dy, :, dx],
                                        start=True, stop=True)
                                sign_evict(ps[:], sqt[:],
                                           b2_t[:, mt:mt + 1],
                                           nb2_t[:, mt:mt + 1])
                                sq2.append(sqt)
                            u01 = tr2pool.tile([128, cimg * 64], BF16,
                                               tag="tr2")
                            u23 = tr2pool.tile([128, cimg * 64], BF16,
                                               tag="tr2")
                            nc.vector.tensor_max(u01[:], sq2[0][:], sq2[1][:])
                            nc.vector.tensor_max(u23[:], sq2[2][:], sq2[3][:])
                            h2v = h2t[mt][:].rearrange(
                                "p (s b) -> p b s", b=nb)
                            nc.vector.tensor_max(
                                h2v[:, g * cimg:(g + 1) * cimg, :],
                                u01[:].rearrange("p (b s) -> p b s", b=cimg),
                                u23[:].rearrange("p (b s) -> p b s", b=cimg))
        # ship own shard to DRAM for the collective
        for mt in range(2):
            nc.sync.dma_start(h2_shard[mt], h2t[mt][:])

      # ---- AllGather (raw phase; Tile-emitted collectives break codegen) ----
      if ncores > 1:
          with nc.Block() as blk, nc.semaphore(f"cc_sem_{_rep}") as cc_sem:
              @blk.gpsimd
              def _(gp):
                  gp.collective_compute(
                      "AllGather", ALU.bypass,
                      replica_groups=[list(range(ncores))],
                      ins=[h2_shard], outs=[h2_all],
                  ).then_inc(cc_sem)
                  gp.wait_ge(cc_sem, 1)
          nc.all_engine_barrier()
      else:
          with nc.Block() as blk, nc.semaphore(f"cp_sem_{_rep}") as cp_sem:
              @blk.gpsimd
              def _(gp):
                  gp.dma_start(h2_all[0], h2_shard[:]).then_inc(cp_sem, 16)
                  gp.wait_ge(cp_sem, 16)
          nc.all_engine_barrier()

      # ---- fc1 + fc2 (Tile phase 2) ----
      with tile.TileContext(nc) as tc2, ExitStack() as ctx2:
          hgp = ctx2.enter_context(tc2.tile_pool(name="hg", bufs=1))
          wp = ctx2.enter_context(tc2.tile_pool(name="wfc", bufs=6))
          sp = ctx2.enter_context(tc2.tile_pool(name="fc1out", bufs=1))
          psp = ctx2.enter_context(tc2.tile_pool(name="fcps", bufs=2,
                                                 space="PSUM"))
          p10 = ctx2.enter_context(tc2.tile_pool(name="fc2ps", bufs=2,
                                                 space="PSUM"))
          yp = ctx2.enter_context(tc2.tile_pool(name="yout", bufs=1))

          hg = {}
          for ct in range(2):
              for bc in range(nbc):
                  t = hgp.tile([128, sh_per_bc * nb * 64], FP8,
                               tag=f"hg{ct}{bc}", name=f"hg{ct}{bc}")
                  for s in range(sh_per_bc):
                      nc.sync.dma_start(
                          t[:, s * nb * 64:(s + 1) * nb * 64],
                          h2_all[bc * sh_per_bc + s, ct])
                  hg[(ct, bc)] = t

          import os as _os
          if _os.environ.get("BCN_DEBUG"):
              dbg = nc.dram_tensor(f"dbg_hg_{_rep}", [2, nbc, 128,
                                   sh_per_bc * nb * 64], FP8,
                                   kind="ExternalOutput").ap()
              for ct in range(2):
                  for bc in range(nbc):
                      nc.sync.dma_start(dbg[ct, bc], hg[(ct, bc)][:])
              dbg2 = nc.dram_tensor(f"dbg_s1_{_rep}", [128, nball], FP16,
                                    kind="ExternalOutput").ap()
          s1 = sp.tile([128, nball], FP16)
          psf = [psp.tile([128, bc_n], F32, tag=f"psf{bc}", name=f"psf{bc}")
                 for bc in range(nbc)]
          f2hi_t = sp.tile([128, 10], FP16)
          nc.sync.dma_start(f2hi_t[:], f2hi[:])
          f2lo_t = sp.tile([128, 10], FP16)
          nc.sync.dma_start(f2lo_t[:], f2lo[:])

          for kt in range(NKT):
              ct, s0 = kt // 64, kt % 64
              whit = wp.tile([128, FPC], FP16, tag="w")
              nc.sync.dma_start(whit[:], whi[kt])
              wlot = wp.tile([128, FPC], FP16, tag="w")
              nc.sync.dma_start(wlot[:], wlo[kt])
              for bc in range(nbc):
                  rhs = hg[(ct, bc)][:].rearrange(
                      "p (s x b) -> p s x b", s=sh_per_bc, b=nb)[:, :, s0, :]
                  nc.tensor.matmul(psf[bc][:], whit[:], rhs,
                                   start=(kt == 0), stop=False)
                  nc.tensor.matmul(psf[bc][:], wlot[:], rhs,
                                   start=False, stop=(kt == NKT - 1))

          for bc in range(nbc):
              nc.scalar.activation(s1[:, bc * bc_n:(bc + 1) * bc_n],
                                   psf[bc][:], AF.Sign)

          if _os.environ.get("BCN_DEBUG"):
              nc.sync.dma_start(dbg2[:], s1[:])
              dbg3 = nc.dram_tensor(f"dbg_psf_{_rep}", [nbc, 128, bc_n], F32,
                                    kind="ExternalOutput").ap()
              for bc in range(nbc):
                  dtmp = sp.tile([128, bc_n], F32, tag=f"dbg{bc}",
                                 name=f"dbg{bc}")
                  nc.vector.tensor_copy(dtmp[:], psf[bc][:])
                  nc.sync.dma_start(dbg3[bc], dtmp[:])
          yt = yp.tile([10, nball], F32)
          for bc in range(nbc):
              ps10 = p10.tile([10, bc_n], F32, tag="ps10")
              nc.tensor.matmul(ps10[:], f2hi_t[:],
                               s1[:, bc * bc_n:(bc + 1) * bc_n],
                               start=True, stop=False)
              nc.tensor.matmul(ps10[:], f2lo_t[:],
                               s1[:, bc * bc_n:(bc + 1) * bc_n],
                               start=False, stop=True)
              nc.scalar.copy(yt[:, bc * bc_n:(bc + 1) * bc_n], ps10[:])
          nc.sync.dma_start(y_out[:], yt[:])

    nc.compile()
    return nc


_CACHE = {}


def _get_program(ncores=NCORES, nb=B):
    key = (ncores, nb)
    if key not in _CACHE:
        _CACHE[key] = build_program(ncores, nb)
    return _CACHE[key]


def kernel(**inputs):
    per_core = _host_prep(**inputs)
    nc = _get_program()
    res = run_bass_kernel_spmd(nc, per_core, core_ids=list(range(NCORES)))
    fc2_b = np.asarray(inputs["fc2_b"], np.float32)
    y = np.zeros((10, NB_ALL), np.float32)
    for n in range(NCORES):
        y += res.results[n]["y"]
    return (y.T + fc2_b[None, :]).astype(np.float32)

